# revision 5
# baseline (speedup 1.0000x reference)
"""Density-aware Chamfer loss on 8 Trainium2 NeuronCores.

Sharding: batch dim B=8, one batch element per core (data parallel).
Per core, for its [4096,3] pred/gt clouds:
  - TensorE computes all pairwise squared distances of the three matrices
    (pred-pred, gt-gt, pred-gt) as augmented K=24 bf16 matmuls:
    d_ij = |a_i|^2 + |b_j|^2 - 2 a_i.b_j with coordinates split into
    bf16 hi/mid/lo triples so the product set reproduces fp32-grade
    accuracy and the self-distance cancels to exactly zero.
  - ScalarE applies exp(-0.5 d / bw^2) with a fused free-axis sum
    (accum_out) -> KDE density row sums for pred-pred and gt-gt.
  - VectorE handles the chamfer mins of pred-gt: tensor_tensor_reduce
    copies each PSUM tile to bf16 SBUF with a fused row-min (pred->gt),
    and a bf16 tensor_tensor min folds row-tiles for the gt->pred
    direction (column mins).
Host gathers per-core partials and does the O(B*N) tail math.
"""

import numpy as np
import ml_dtypes
from contextlib import ExitStack

import concourse.bacc as bacc
import concourse.tile as tile
from concourse import mybir
from concourse.bass_utils import run_bass_kernel_spmd

B = 8
N = 4096
N_CORES = 8
PB = 128          # rows per m-tile (PSUM partitions)
NMT = N // PB     # 32 m-tiles
GW = 1024         # columns per consumer group (2 PSUM banks)
NG = N // GW      # 4 groups per m-tile row
MMW = 512         # matmul moving free dim
K = 24            # augmented contraction depth
KP = 32           # padded partition alloc for feature arrays
EPS = 1e-6

_BF16 = ml_dtypes.bfloat16
_BIG = 3.0e38


def _split3(x):
    """x float64 -> (h, m, l) float64 arrays whose values are exactly
    representable in bf16 and sum to x within ~2^-24 relative."""
    h = x.astype(_BF16).astype(np.float64)
    r = x - h
    m = r.astype(_BF16).astype(np.float64)
    l = (r - m).astype(_BF16).astype(np.float64)
    return h, m, l


# product pairs over (hi, mid, lo) kept in the augmented matmul
_PAIRS = [(0, 0), (0, 1), (1, 0), (0, 2), (2, 0), (1, 1)]


def _features(pts):
    """pts [N,3] float64 -> (U, V) bf16 arrays [KP, N].

    Row k of U (as lhsT) times row k of V (as rhs) summed over k gives
    the pairwise squared distance  |a_i - b_j|^2  (up to ~2^-24), with
    the diagonal of a same-cloud product exactly zero.
    """
    n = pts.shape[0]
    sp = [np.stack(_split3(pts[:, c])) for c in range(3)]  # 3 x [3, N]
    U = np.zeros((KP, n))
    V = np.zeros((KP, n))
    x2 = np.zeros(n)
    r = 0
    for c in range(3):
        s = sp[c]
        for iu, iv in _PAIRS:
            U[r] = s[iu]
            V[r] = -2.0 * s[iv]
            x2 += s[iu] * s[iv]
            r += 1
    h, m, l = _split3(x2)
    U[18], U[19], U[20] = h, m, l
    V[18:21] = 1.0
    U[21:24] = 1.0
    V[21], V[22], V[23] = h, m, l
    return U.astype(_BF16), V.astype(_BF16)


def _build(gamma_p, gamma_g):
    """Build + bacc-compile the 8-core SPMD program."""
    nc = bacc.Bacc(
        "TRN2", target_bir_lowering=False, debug=False, num_devices=N_CORES
    )
    f32 = mybir.dt.float32
    bf16 = mybir.dt.bfloat16
    Exp = mybir.ActivationFunctionType.Exp
    Min = mybir.AluOpType.min

    u_pred = nc.dram_tensor("u_pred", [KP, N], bf16, kind="ExternalInput").ap()
    v_pred = nc.dram_tensor("v_pred", [KP, N], bf16, kind="ExternalInput").ap()
    u_gt = nc.dram_tensor("u_gt", [KP, N], bf16, kind="ExternalInput").ap()
    v_gt = nc.dram_tensor("v_gt", [KP, N], bf16, kind="ExternalInput").ap()

    minp_o = nc.dram_tensor("minp", [PB, NMT * NG], f32, kind="ExternalOutput").ap()
    densp_o = nc.dram_tensor("densp", [PB, NMT * NG], f32, kind="ExternalOutput").ap()
    densg_o = nc.dram_tensor("densg", [PB, NMT * NG], f32, kind="ExternalOutput").ap()
    colrun_o = nc.dram_tensor("colrun", [PB, N], bf16, kind="ExternalOutput").ap()

    with tile.TileContext(nc) as tc:
        with ExitStack() as ctx:
            feat = ctx.enter_context(tc.tile_pool(name="feat", bufs=1))
            persist = ctx.enter_context(tc.tile_pool(name="persist", bufs=1))
            scr = ctx.enter_context(tc.tile_pool(name="scr", bufs=2))
            raw_pool = ctx.enter_context(tc.tile_pool(name="rawp", bufs=2))
            pp_pool = ctx.enter_context(tc.tile_pool(name="ppp", bufs=1, space="PSUM"))
            gg_pool = ctx.enter_context(tc.tile_pool(name="ggp", bufs=1, space="PSUM"))
            pg_pool = ctx.enter_context(tc.tile_pool(name="pgp", bufs=2, space="PSUM"))

            Up = feat.tile([KP, N], bf16)
            nc.sync.dma_start(Up[:], u_pred[:])
            Vp = feat.tile([KP, N], bf16)
            nc.sync.dma_start(Vp[:], v_pred[:])
            Ug = feat.tile([KP, N], bf16)
            nc.sync.dma_start(Ug[:], u_gt[:])
            Vg = feat.tile([KP, N], bf16)
            nc.sync.dma_start(Vg[:], v_gt[:])

            minp_t = persist.tile([PB, NMT * NG], f32)
            densp_t = persist.tile([PB, NMT * NG], f32)
            densg_t = persist.tile([PB, NMT * NG], f32)
            colrun = persist.tile([PB, N], bf16)

            for mt in range(NMT):
                lhs_p = Up[0:K, mt * PB:(mt + 1) * PB]
                lhs_g = Ug[0:K, mt * PB:(mt + 1) * PB]
                for g in range(NG):
                    col0 = g * GW
                    pcol = mt * NG + g

                    pp = pp_pool.tile([PB, GW], f32)
                    pg = pg_pool.tile([PB, GW], f32)
                    gg = gg_pool.tile([PB, GW], f32)
                    for h in range(GW // MMW):
                        c = col0 + h * MMW
                        nc.tensor.matmul(
                            pp[:, h * MMW:(h + 1) * MMW],
                            lhs_p, Vp[0:K, c:c + MMW], start=True, stop=True,
                        )
                    for h in range(GW // MMW):
                        c = col0 + h * MMW
                        nc.tensor.matmul(
                            pg[:, h * MMW:(h + 1) * MMW],
                            lhs_p, Vg[0:K, c:c + MMW], start=True, stop=True,
                        )
                    for h in range(GW // MMW):
                        c = col0 + h * MMW
                        nc.tensor.matmul(
                            gg[:, h * MMW:(h + 1) * MMW],
                            lhs_g, Vg[0:K, c:c + MMW], start=True, stop=True,
                        )

                    # density rows: exp + fused row-sum
                    scr_pp = scr.tile([PB, GW], bf16, tag="scr")
                    nc.scalar.activation(
                        scr_pp[:], pp[:], Exp, scale=-gamma_p,
                        accum_out=densp_t[:, pcol:pcol + 1],
                    )

                    # chamfer pred->gt: PSUM -> bf16 copy + fused row-min;
                    # gt->pred: fold into colrun
                    if mt == 0:
                        nc.vector.tensor_scalar(
                            colrun[:, col0:col0 + GW], pg[:], _BIG, None,
                            Min, Min, accum_out=minp_t[:, pcol:pcol + 1],
                        )
                    else:
                        raw = raw_pool.tile([PB, GW], bf16, tag="raw")
                        nc.vector.tensor_scalar(
                            raw[:], pg[:], _BIG, None,
                            Min, Min, accum_out=minp_t[:, pcol:pcol + 1],
                        )
                        nc.vector.tensor_tensor(
                            colrun[:, col0:col0 + GW],
                            colrun[:, col0:col0 + GW], raw[:], Min,
                        )

                    scr_gg = scr.tile([PB, GW], bf16, tag="scr")
                    nc.scalar.activation(
                        scr_gg[:], gg[:], Exp, scale=-gamma_g,
                        accum_out=densg_t[:, pcol:pcol + 1],
                    )

            nc.sync.dma_start(minp_o[:], minp_t[:])
            nc.sync.dma_start(densp_o[:], densp_t[:])
            nc.sync.dma_start(densg_o[:], densg_t[:])
            nc.sync.dma_start(colrun_o[:], colrun[:])

    nc.compile()
    return nc


_CACHE = {}


def _get_program(gamma_p, gamma_g):
    key = (float(gamma_p), float(gamma_g))
    if key not in _CACHE:
        _CACHE[key] = _build(*key)
    return _CACHE[key]


def kernel(predicted, ground_truth, bandwidth_pred, bandwidth_gt, _trace=False):
    predicted = np.asarray(predicted, dtype=np.float32)
    ground_truth = np.asarray(ground_truth, dtype=np.float32)
    bw_p = float(np.asarray(bandwidth_pred))
    bw_g = float(np.asarray(bandwidth_gt))
    gamma_p = 0.5 / (bw_p * bw_p)
    gamma_g = 0.5 / (bw_g * bw_g)

    nc = _get_program(gamma_p, gamma_g)

    in_maps = []
    for b in range(B):
        Upb, Vpb = _features(predicted[b].astype(np.float64))
        Ugb, Vgb = _features(ground_truth[b].astype(np.float64))
        in_maps.append({"u_pred": Upb, "v_pred": Vpb, "u_gt": Ugb, "v_gt": Vgb})

    res = run_bass_kernel_spmd(
        nc, in_maps, core_ids=list(range(N_CORES)), trace=_trace
    )

    total_p = 0.0
    total_g = 0.0
    for b in range(B):
        r = res.results[b]
        minp = r["minp"].reshape(PB, NMT, NG).min(axis=2).T.reshape(-1)
        densp = r["densp"].astype(np.float64).reshape(PB, NMT, NG).sum(axis=2)
        densp = densp.T.reshape(-1) / (N - 1)
        densg = r["densg"].astype(np.float64).reshape(PB, NMT, NG).sum(axis=2)
        densg = densg.T.reshape(-1) / (N - 1)
        ming = r["colrun"].astype(np.float32).min(axis=0)
        total_p += (minp.astype(np.float64) / (densp + EPS)).sum()
        total_g += (ming.astype(np.float64) / (densg + EPS)).sum()

    loss = total_p / (B * N) + total_g / (B * N)
    if _trace:
        kernel._last_results = res
    return np.float32(loss)


# revision 17
# speedup vs baseline: 1.0266x; 1.0266x over previous
"""Density-aware Chamfer loss on 8 Trainium2 NeuronCores.

Sharding: batch dim B=8, one batch element per core (data parallel).
Per core, for its [4096,3] pred/gt clouds:
  - TensorE computes pairwise squared distances as augmented K=24 bf16
    matmuls: d_ij = |a_i|^2 + |b_j|^2 - 2 a_i.b_j with coordinates split
    into bf16 hi/mid/lo triples so the product set reproduces fp32-grade
    accuracy and the self-distance cancels to exactly zero.
  - Density matrices (pred-pred, gt-gt) are symmetric: only tiles with
    strip >= block(m-tile) are computed.  ScalarE applies
    exp(-0.5 d / bw^2) with a fused free-axis sum (accum_out) giving row
    sums of computed tiles; the missing lower-triangle row sums are
    recovered as column sums of the strictly-upper tiles via ones-matmuls
    on TensorE, accumulated per strip in PSUM (4 chains per bank via
    col-group tile_position).
  - VectorE handles the chamfer mins of pred-gt: tensor_scalar
    (min with +BIG, fused min-reduce accum_out) copies each PSUM tile to
    bf16 SBUF with the row min for pred->gt, and a bf16 tensor_tensor
    min folds row tiles for the gt->pred direction (column mins).
Host gathers per-core partials and does the O(B*N) tail math.
"""

import numpy as np
import ml_dtypes
from contextlib import ExitStack

import concourse.bacc as bacc
import concourse.tile as tile
from concourse import mybir
from concourse.bass_utils import run_bass_kernel_spmd

B = 8
N = 4096
N_CORES = 8
PB = 128          # rows per m-tile (PSUM partitions)
NMT = N // PB     # 32 m-tiles
SW = 512          # strip width (one PSUM bank, matmul moving max)
NS = N // SW      # 8 strips
GW = 1024         # chamfer consumer group width (2 PSUM banks)
NG = N // GW      # 4 chamfer groups per m-tile row
K = 24            # augmented contraction depth
KP = 32           # padded partition alloc for feature arrays
EPS = 1e-6

_BF16 = ml_dtypes.bfloat16
_BIG = 3.0e38


def _split3(x):
    """x float64 -> (h, m, l) float64 arrays whose values are exactly
    representable in bf16 and sum to x within ~2^-24 relative."""
    h = x.astype(_BF16).astype(np.float64)
    r = x - h
    m = r.astype(_BF16).astype(np.float64)
    l = (r - m).astype(_BF16).astype(np.float64)
    return h, m, l


# product pairs over (hi, mid, lo) kept in the augmented matmul
_PAIRS = [(0, 0), (0, 1), (1, 0), (0, 2), (2, 0), (1, 1)]


def _features(pts):
    """pts [N,3] float64 -> (U, V) bf16 arrays [KP, N].

    Row k of U (as lhsT) times row k of V (as rhs) summed over k gives
    the pairwise squared distance  |a_i - b_j|^2  (up to ~2^-24), with
    the diagonal of a same-cloud product exactly zero.
    """
    n = pts.shape[0]
    sp = [np.stack(_split3(pts[:, c])) for c in range(3)]  # 3 x [3, N]
    U = np.zeros((KP, n))
    V = np.zeros((KP, n))
    x2 = np.zeros(n)
    r = 0
    for c in range(3):
        s = sp[c]
        for iu, iv in _PAIRS:
            U[r] = s[iu]
            V[r] = -2.0 * s[iv]
            x2 += s[iu] * s[iv]
            r += 1
    h, m, l = _split3(x2)
    U[18], U[19], U[20] = h, m, l
    V[18:21] = 1.0
    U[21:24] = 1.0
    V[21], V[22], V[23] = h, m, l
    return U.astype(_BF16), V.astype(_BF16)


def _build(gamma_p, gamma_g):
    """Build + bacc-compile the 8-core SPMD program."""
    nc = bacc.Bacc(
        "TRN2", target_bir_lowering=False, debug=False, num_devices=N_CORES
    )
    f32 = mybir.dt.float32
    bf16 = mybir.dt.bfloat16
    Exp = mybir.ActivationFunctionType.Exp
    Min = mybir.AluOpType.min

    u_pred = nc.dram_tensor("u_pred", [KP, N], bf16, kind="ExternalInput").ap()
    v_pred = nc.dram_tensor("v_pred", [KP, N], bf16, kind="ExternalInput").ap()
    u_gt = nc.dram_tensor("u_gt", [KP, N], bf16, kind="ExternalInput").ap()
    v_gt = nc.dram_tensor("v_gt", [KP, N], bf16, kind="ExternalInput").ap()

    minp_o = nc.dram_tensor("minp", [PB, NMT * NG], f32, kind="ExternalOutput").ap()
    densp_o = nc.dram_tensor("densp", [PB, NMT * NS], f32, kind="ExternalOutput").ap()
    densg_o = nc.dram_tensor("densg", [PB, NMT * NS], f32, kind="ExternalOutput").ap()
    colrun_o = nc.dram_tensor("colrun", [PB, N], bf16, kind="ExternalOutput").ap()
    # column-sum chains: rows 0..6 = pred strips 1..7, rows 7..13 = gt
    csum_o = nc.dram_tensor("csum", [1, 14 * SW], f32, kind="ExternalOutput").ap()

    with tile.TileContext(nc) as tc:
        with ExitStack() as ctx:
            feat = ctx.enter_context(tc.tile_pool(name="feat", bufs=1))
            persist = ctx.enter_context(tc.tile_pool(name="persist", bufs=1))
            scr = ctx.enter_context(tc.tile_pool(name="scr", bufs=16))
            raw_pool = ctx.enter_context(tc.tile_pool(name="rawp", bufs=2))
            den_pool = ctx.enter_context(tc.tile_pool(name="denp", bufs=2, space="PSUM"))
            pg_pool = ctx.enter_context(tc.tile_pool(name="pgp", bufs=1, space="PSUM"))
            cs_pool = ctx.enter_context(tc.tile_pool(name="csp", bufs=1, space="PSUM"))

            Up = feat.tile([KP, N], bf16)
            nc.sync.dma_start(Up[:], u_pred[:])
            Vp = feat.tile([KP, N], bf16)
            nc.sync.dma_start(Vp[:], v_pred[:])
            Ug = feat.tile([KP, N], bf16)
            nc.sync.dma_start(Ug[:], u_gt[:])
            Vg = feat.tile([KP, N], bf16)
            nc.sync.dma_start(Vg[:], v_gt[:])

            ones_t = feat.tile([PB, 1], bf16)
            nc.vector.memset(ones_t[:], 1.0)

            minp_t = persist.tile([PB, NMT * NG], f32)
            densp_t = persist.tile([PB, NMT * NS], f32)
            nc.vector.memset(densp_t[:], 0.0)
            densg_t = persist.tile([PB, NMT * NS], f32)
            nc.vector.memset(densg_t[:], 0.0)
            colrun = persist.tile([PB, N], bf16)

            # colsum chains: strips 1..3 in bank cs[mat][0] at partitions
            # 32*s, strips 4..7 in bank cs[mat][1] at partitions 32*(s-4).
            cs = {}
            for mat in ("p", "g"):
                cs[mat] = [
                    cs_pool.tile([PB, SW], f32, name=f"cs_{mat}0"),
                    cs_pool.tile([PB, SW], f32, name=f"cs_{mat}1"),
                ]

            # emission order: alternate heavy (low-block) and light
            # (high-block) m-tiles so ACT density work stays level
            mt_order = []
            for i in range(NMT // 2):
                mt_order.append(i)
                mt_order.append(NMT - 1 - i)
            pos = {mt: i for i, mt in enumerate(mt_order)}

            def chain_last(s):
                # last contributor (mt < 4s) in emission order
                return max(range(4 * s), key=lambda m: pos[m])

            def colsum(mat, mt, s, rhs):
                bank = cs[mat][0] if s < 4 else cs[mat][1]
                j = (s % 4) * 32
                nc.tensor.matmul(
                    bank[j:j + 1, :], ones_t[:, 0:1], rhs,
                    start=(mt == 0), stop=(mt == chain_last(s)),
                    tile_position=(0, j), skip_group_check=True,
                )

            for mt in mt_order:
                blk = mt // 4
                lhs_p = Up[0:K, mt * PB:(mt + 1) * PB]
                lhs_g = Ug[0:K, mt * PB:(mt + 1) * PB]

                pg_tiles = []
                for g in range(NG):
                    pg_tiles.append(
                        pg_pool.tile([PB, GW], f32, tag="pg", name=f"pg_{mt}_{g}")
                    )

                scr_tiles = []
                for s in range(NS):
                    # chamfer pred-gt strip matmul (full matrix)
                    pg = pg_tiles[s // 2]
                    nc.tensor.matmul(
                        pg[:, (s % 2) * SW:(s % 2) * SW + SW],
                        lhs_p, Vg[0:K, s * SW:(s + 1) * SW],
                        start=True, stop=True,
                    )
                    if s >= blk:
                        # density tiles (upper + diagonal only)
                        dp = den_pool.tile([PB, SW], f32, tag="den")
                        nc.tensor.matmul(
                            dp[:], lhs_p, Vp[0:K, s * SW:(s + 1) * SW],
                            start=True, stop=True,
                        )
                        dg = den_pool.tile([PB, SW], f32, tag="den")
                        nc.tensor.matmul(
                            dg[:], lhs_g, Vg[0:K, s * SW:(s + 1) * SW],
                            start=True, stop=True,
                        )
                        pcol = mt * NS + s
                        ep = scr.tile([PB, SW], bf16, tag="scr")
                        nc.scalar.activation(
                            ep[:], dp[:], Exp, scale=-gamma_p,
                            accum_out=densp_t[:, pcol:pcol + 1],
                        )
                        eg = scr.tile([PB, SW], bf16, tag="scr")
                        nc.scalar.activation(
                            eg[:], dg[:], Exp, scale=-gamma_g,
                            accum_out=densg_t[:, pcol:pcol + 1],
                        )
                        if s > blk:
                            scr_tiles.append((s, ep, eg))

                    if s % 2 == 1:
                        # chamfer consumers for the completed 1024 group
                        g = s // 2
                        pg = pg_tiles[g]
                        col0 = g * GW
                        pcol = mt * NG + g
                        if mt == 0:
                            nc.vector.tensor_scalar(
                                colrun[:, col0:col0 + GW], pg[:], _BIG, None,
                                Min, Min, accum_out=minp_t[:, pcol:pcol + 1],
                            )
                        else:
                            raw = raw_pool.tile([PB, GW], bf16, tag="raw")
                            nc.vector.tensor_scalar(
                                raw[:], pg[:], _BIG, None,
                                Min, Min, accum_out=minp_t[:, pcol:pcol + 1],
                            )
                            nc.vector.tensor_tensor(
                                colrun[:, col0:col0 + GW],
                                colrun[:, col0:col0 + GW], raw[:], Min,
                            )

                # column-sum matmuls for this m-tile's strictly-upper tiles
                for s, ep, eg in scr_tiles:
                    colsum("p", mt, s, ep[:])
                    colsum("g", mt, s, eg[:])

            # copy colsum chain rows PSUM -> SBUF staging, then DMA out
            cs_stage = persist.tile([1, 14 * SW], f32)
            for i, mat in enumerate(("p", "g")):
                for s in range(1, NS):
                    bank = cs[mat][0] if s < 4 else cs[mat][1]
                    j = (s % 4) * 32
                    row = 7 * i + (s - 1)
                    nc.scalar.copy(
                        cs_stage[0:1, row * SW:(row + 1) * SW],
                        bank[j:j + 1, :],
                    )
            nc.sync.dma_start(csum_o[:], cs_stage[:])

            nc.sync.dma_start(minp_o[:], minp_t[:])
            nc.sync.dma_start(densp_o[:], densp_t[:])
            nc.sync.dma_start(densg_o[:], densg_t[:])
            nc.sync.dma_start(colrun_o[:], colrun[:])

    nc.compile()
    return nc


_CACHE = {}


def _get_program(gamma_p, gamma_g):
    key = (float(gamma_p), float(gamma_g))
    if key not in _CACHE:
        _CACHE[key] = _build(*key)
    return _CACHE[key]


def kernel(predicted, ground_truth, bandwidth_pred, bandwidth_gt, _trace=False):
    predicted = np.asarray(predicted, dtype=np.float32)
    ground_truth = np.asarray(ground_truth, dtype=np.float32)
    bw_p = float(np.asarray(bandwidth_pred))
    bw_g = float(np.asarray(bandwidth_gt))
    gamma_p = 0.5 / (bw_p * bw_p)
    gamma_g = 0.5 / (bw_g * bw_g)

    nc = _get_program(gamma_p, gamma_g)

    in_maps = []
    for b in range(B):
        Upb, Vpb = _features(predicted[b].astype(np.float64))
        Ugb, Vgb = _features(ground_truth[b].astype(np.float64))
        in_maps.append({"u_pred": Upb, "v_pred": Vpb, "u_gt": Ugb, "v_gt": Vgb})

    res = run_bass_kernel_spmd(
        nc, in_maps, core_ids=list(range(N_CORES)), trace=_trace
    )

    total_p = 0.0
    total_g = 0.0
    for b in range(B):
        r = res.results[b]
        minp = r["minp"].reshape(PB, NMT, NG).min(axis=2).T.reshape(-1)
        ming = r["colrun"].astype(np.float32).min(axis=0)
        csum = r["csum"].reshape(14, SW)
        densp = _assemble_density(r["densp"], csum[0:7])
        densg = _assemble_density(r["densg"], csum[7:14])
        total_p += (minp.astype(np.float64) / (densp + EPS)).sum()
        total_g += (ming.astype(np.float64) / (densg + EPS)).sum()

    loss = total_p / (B * N) + total_g / (B * N)
    if _trace:
        kernel._last_results = res
    return np.float32(loss)


def _assemble_density(parts, csum):
    """parts [128, NMT*NS] f32 (valid only where s >= mt//4);
    csum [7, SW]: row s-1 = accumulated column sums of strip s."""
    parts = parts.astype(np.float64).reshape(PB, NMT, NS)
    dens = np.zeros(N)
    for mt in range(NMT):
        blk = mt // 4
        rows = parts[:, mt, blk:].sum(axis=1)  # [128]
        dens[mt * PB:(mt + 1) * PB] = rows
    for s in range(1, NS):
        dens[s * SW:(s + 1) * SW] += csum[s - 1].astype(np.float64)
    return dens / (N - 1)
